# revision 35
# baseline (speedup 1.0000x reference)
"""Distributed multi-head attention kernel for one TRN2 chip (8 NeuronCores).

Problem: x[4, 2048, 1024] -> qkv Linear(1024, 3072, bias=False) -> 16-head
softmax attention -> proj Linear(1024, 1024) + bias.

Sharding: tensor-parallel over heads. Core c owns heads {2c, 2c+1} (128 of the
1024 qkv feature dims). Each core computes Q/K/V for its head pair over the
full sequence, runs attention per (batch, head), then reshards with one
AllToAll per HALF batch (1024 tokens) so core c ends up with the full 1024
attention features for a 128-token slice of each half. Each core then applies
the full W_proj to its token slices and the host scatters the 8 shards.

Engine schedule (the point of this version): the ScalarEngine exp stream
(~1.38us per k-tile) and the TensorEngine are co-bottlenecks, and the PE's
DVFS drops to half clock on every pipeline bubble. So the emission interleaves
at k-tile granularity: step k of chunk c's score matmuls is paired with step
k-2 of chunk c-1's PV matmuls (the 2-step offset gives the PV PSUM drain a
window before the next chunk's PV accumulation reuses the bank). QKV/proj
matmuls are woven in as ACT-independent ballast: V for batch b inside the
first chunk of batch b, Q/K as a block at the previous batch's end, proj
halves at chunk boundaries two chunks after their collective was issued.

Layout notes:
 - x is transposed on the host to xT [C, B*N] so SBUF tiles have the
   contraction dim (C) on partitions for the QKV matmuls.
 - Q and K are produced transposed (QT/KT [128 head-dims, tokens]) which is
   exactly the operand layout for S^T = K Q^T. Scores are built transposed
   (ST [k_tok, q_tok]) so that P^T is directly the lhs-side operand of the
   PV matmul (k_tok on partitions).
 - V is produced in natural [token, head-dim] layout with an extra all-ones
   column per head, so the PV matmul also yields the softmax denominator row
   (row 64 of the [65, q] output) for free.
 - No row-max subtraction: scores are ~N(0,1) after scaling so exp is safe.
"""

import os
import sys

import numpy as np

for _p in ("/opt/trn_rl_repo", "/root/.axon_site/_ro/trn_rl_repo"):
    if os.path.isdir(_p) and _p not in sys.path:
        sys.path.append(_p)

import ml_dtypes  # noqa: E402

B, N, C = 4, 2048, 1024
NUM_HEADS = 16
HEAD_DIM = C // NUM_HEADS  # 64
SCALE = HEAD_DIM**-0.5
NCORES = 8
P = 128  # SBUF partitions
QC = 512  # q-chunk (matmul free dim / PSUM bank)
TPH = 128  # tokens per core per half-batch after reshard

BF16 = ml_dtypes.bfloat16


def build_attention_nc(NB: int = B, NQ: int = N, CH: int = C):
    """Build + compile the SPMD graph. NB batches of NQ tokens, CH channels."""
    import concourse.bass as bass
    import concourse.mybir as mybir
    import concourse.tile as tile
    from concourse import bacc

    f32 = mybir.dt.float32
    bf16 = mybir.dt.bfloat16

    n_qc = NQ // QC          # q chunks per batch (4)
    n_kt = NQ // P           # k tiles per batch (16)
    n_cc = CH // P           # contraction chunks (8)
    n_ck = NB * n_qc         # total chunks (16)

    nc = bacc.Bacc("TRN2", target_bir_lowering=False, debug=False,
                   num_devices=NCORES)

    xT = nc.dram_tensor("xT", [CH, NB * NQ], bf16, kind="ExternalInput").ap()
    wq = nc.dram_tensor("wq", [CH, P], bf16, kind="ExternalInput").ap()
    wk = nc.dram_tensor("wk", [CH, P], bf16, kind="ExternalInput").ap()
    wv = nc.dram_tensor("wv", [CH, P], bf16, kind="ExternalInput").ap()
    wp = nc.dram_tensor("wp", [CH, CH], bf16, kind="ExternalInput").ap()
    bp = nc.dram_tensor("bp", [1, CH], f32, kind="ExternalInput").ap()
    ident = nc.dram_tensor("ident", [P, P], bf16, kind="ExternalInput").ap()
    out = nc.dram_tensor("out", [NB * 2 * TPH, CH], f32,
                         kind="ExternalOutput").ap()

    from contextlib import ExitStack

    with tile.TileContext(nc) as tc, ExitStack() as ctx:
        const = ctx.enter_context(tc.tile_pool(name="const", bufs=1))
        xt_pool = ctx.enter_context(tc.tile_pool(name="xt", bufs=9))
        qk_pool = ctx.enter_context(tc.tile_pool(name="qk", bufs=2))
        v_pool = ctx.enter_context(tc.tile_pool(name="v", bufs=2))
        pt_pool = ctx.enter_context(tc.tile_pool(name="pt", bufs=2))
        ot_pool = ctx.enter_context(tc.tile_pool(name="ot", bufs=2))
        num_pool = ctx.enter_context(tc.tile_pool(name="num", bufs=2))
        div_pool = ctx.enter_context(tc.tile_pool(name="div", bufs=2))
        at_pool = ctx.enter_context(tc.tile_pool(name="at", bufs=16))
        y_pool = ctx.enter_context(tc.tile_pool(name="y", bufs=2))
        dram = ctx.enter_context(tc.tile_pool(name="dram", bufs=1, space="DRAM"))
        ps_st = ctx.enter_context(tc.tile_pool(name="ps_st", bufs=2, space="PSUM"))
        ps_pv = ctx.enter_context(tc.tile_pool(name="ps_pv", bufs=1, space="PSUM"))
        ps_mm = ctx.enter_context(tc.tile_pool(name="ps_mm", bufs=2, space="PSUM"))

        # --- resident weights ---
        # wq/wk/wv go on the gpsimd software-DGE queue so they don't contend
        # with the first batch's x^T tiles on the sync DMA queue.
        wq_sb = const.tile([P, n_cc, P], bf16, tag="wq")
        wk_sb = const.tile([P, n_cc, P], bf16, tag="wk")
        wv_sb = const.tile([P, n_cc, P], bf16, tag="wv")
        nc.gpsimd.dma_start(wq_sb[:], wq.rearrange("(cc p) m -> p cc m", p=P))
        nc.gpsimd.dma_start(wk_sb[:], wk.rearrange("(cc p) m -> p cc m", p=P))
        nc.gpsimd.dma_start(wv_sb[:], wv.rearrange("(cc p) m -> p cc m", p=P))
        wp_sb = const.tile([P, n_cc, CH], bf16, tag="wp")
        bias_row = const.tile([1, CH], f32, tag="bias_row")
        bias_sb = const.tile([P, CH], f32, tag="bias")
        ident_sb = const.tile([P, P], bf16, tag="ident")
        nc.gpsimd.dma_start(ident_sb[:], ident[:, :])

        # Pre-warm the ACT exp table during the initial DMA window so the
        # first real exp doesn't eat the ~2.7us table load.
        warm = const.tile([1, 2], f32, tag="warm")
        nc.vector.memset(warm[:, 0:1], 0.0)
        nc.scalar.activation(warm[:, 1:2], warm[:, 0:1],
                             mybir.ActivationFunctionType.Exp)

        # Pre-warm the collective engine the same way: the first real
        # AllToAll otherwise pays a ~50us one-time CC init that stalls the
        # first projection's at-loads mid-stream.
        cc_warm_in = dram.tile([NCORES, 32], bf16, tag="ccw_i", name="ccw_i")
        cc_warm_out = dram.tile([NCORES, 32], bf16, tag="ccw_o", name="ccw_o")
        nc.gpsimd.collective_compute(
            "AllToAll", mybir.AluOpType.bypass,
            replica_groups=[list(range(NCORES))],
            ins=[cc_warm_in[:].opt()], outs=[cc_warm_out[:].opt()])

        a2a_in = [[None, None] for _ in range(NB)]
        a2a_out = [[None, None] for _ in range(NB)]
        for b in range(NB):
            for h in range(2):
                a2a_in[b][h] = dram.tile([NCORES * P, TPH], bf16,
                                         tag=f"a2a_in{b}{h}",
                                         name=f"a2a_in{b}{h}")
                a2a_out[b][h] = dram.tile([NCORES * P, TPH], bf16,
                                          tag=f"a2a_out{b}{h}",
                                          name=f"a2a_out{b}{h}")

        # per-batch state
        xts = {}      # b -> list of n_cc xt tiles
        qts, kts, vts = {}, {}, {}
        v_sbs = {}
        ots = {}
        pts = {}      # chunk c -> pt tile
        pvps = {}     # chunk c -> pv psum tile

        def emit_xt(b, split=False):
            tiles = [xt_pool.tile([P, NQ], bf16, tag="xt", name="xt_tile")
                     for _ in range(n_cc)]
            if split:
                # quarter-token loads ordered so the first QK chain's
                # operands (cc-minor within token-half pairs) land first
                hw = NQ // 4
                for hp in range(2):
                    for cc in range(n_cc):
                        for hv in (2 * hp, 2 * hp + 1):
                            nc.sync.dma_start(
                                tiles[cc][:, hv * hw:(hv + 1) * hw],
                                xT[cc * P:(cc + 1) * P,
                                   b * NQ + hv * hw:b * NQ + (hv + 1) * hw])
            else:
                for cc in range(n_cc):
                    nc.sync.dma_start(
                        tiles[cc][:],
                        xT[cc * P:(cc + 1) * P, b * NQ:(b + 1) * NQ])
            xts[b] = tiles

        def qkv_units(b):
            """Q^T / K^T / V^T matmuls for batch b, chopped into 12 ballast
            units of 8 matmuls each (half a psa/psb chain pair). Units 2i and
            2i+1 share PSUM tiles and must be emitted with no other tag-mm
            allocation between them."""
            qt_sb = qk_pool.tile([P, NQ], bf16, tag="qt", name="qt_sb")
            kt_sb = qk_pool.tile([P, NQ], bf16, tag="kt", name="kt_sb")
            vt_sb = v_pool.tile([P, NQ], bf16, tag="vt", name="vt_sb")
            qts[b], kts[b], vts[b] = qt_sb, kt_sb, vt_sb
            units = []
            for qc in range(0, n_qc, 2):
                for w_sb, dst in ((wq_sb, qt_sb), (wk_sb, kt_sb),
                                  (wv_sb, vt_sb)):
                    qsa = slice(qc * QC, (qc + 1) * QC)
                    qsb = slice((qc + 1) * QC, (qc + 2) * QC)
                    st = {}

                    def u1(w_sb=w_sb, qsa=qsa, qsb=qsb, st=st):
                        st["psa"] = ps_mm.tile([P, QC], f32, tag="mm",
                                               name="ps_a")
                        st["psb"] = ps_mm.tile([P, QC], f32, tag="mm",
                                               name="ps_b")
                        for cc in range(n_cc // 2):
                            nc.tensor.matmul(st["psa"][:], w_sb[:, cc, :],
                                             xts[b][cc][:, qsa],
                                             start=(cc == 0), stop=False)
                            nc.tensor.matmul(st["psb"][:], w_sb[:, cc, :],
                                             xts[b][cc][:, qsb],
                                             start=(cc == 0), stop=False)

                    def u2(w_sb=w_sb, dst=dst, qsa=qsa, qsb=qsb, st=st):
                        for cc in range(n_cc // 2, n_cc):
                            nc.tensor.matmul(st["psa"][:], w_sb[:, cc, :],
                                             xts[b][cc][:, qsa],
                                             start=False,
                                             stop=(cc == n_cc - 1))
                            nc.tensor.matmul(st["psb"][:], w_sb[:, cc, :],
                                             xts[b][cc][:, qsb],
                                             start=False,
                                             stop=(cc == n_cc - 1))
                        nc.vector.tensor_copy(dst[:, qsa], st["psa"][:])
                        nc.vector.tensor_copy(dst[:, qsb], st["psb"][:])

                    units += [u1, u2]
            return units

        def alloc_v(b):
            # V natural layout + ones column per head: [tok-tile, 2x(64+1)]
            v_sb = v_pool.tile([P, n_kt, 130], bf16, tag="v", name="v_sb")
            ones_view = v_sb.rearrange("p t (g c) -> p t g c", g=2)[:, :, :, 64:65]
            nc.vector.memset(ones_view, 1.0)
            v_sbs[b] = v_sb

        def emit_v_tile(b, tt):
            """Transpose one 128-token block of V^T back to natural layout."""
            ts_ = slice(tt * P, (tt + 1) * P)
            vps = ps_mm.tile([P, P], bf16, tag="mm", name="vps")
            nc.tensor.transpose(vps[:], vts[b][:, ts_], ident_sb[:])
            dst = v_sbs[b].rearrange("p t (g c) -> p t g c", g=2)[:, tt, :, 0:64]
            nc.vector.tensor_copy(dst, vps.rearrange("p (g c) -> p g c", g=2))

        def emit_scores_step(c, kt):
            """Scores + exp for chunk c, k-tile kt."""
            b, qc = divmod(c, n_qc)
            qs = slice(qc * QC, (qc + 1) * QC)
            ks = slice(kt * P, (kt + 1) * P)
            if kt == 0:
                pts[c] = pt_pool.tile([P, n_kt, 2, QC], bf16, tag="pt",
                                      name="pt_tile")
            st = ps_st.tile([P, 2, QC], f32, tag="st", name="st_tile")
            for h in range(2):
                hs = slice(64 * h, 64 * (h + 1))
                nc.tensor.matmul(st[:, h, :], kts[b][hs, ks], qts[b][hs, qs])
            nc.scalar.activation(pts[c][:, kt, :, :], st[:],
                                 mybir.ActivationFunctionType.Exp, scale=SCALE)

        def emit_pv_step(c, kt):
            b, qc = divmod(c, n_qc)
            if kt == 0:
                pvps[c] = ps_pv.tile([P, 2, QC], f32, tag="pv", name="pv_ps")
            for h in range(2):
                nc.tensor.matmul(
                    pvps[c][0:65, h, :], v_sbs[b][:, kt, 65 * h:65 * (h + 1)],
                    pts[c][:, kt, h, :],
                    start=(kt == 0), stop=(kt == n_kt - 1))

        def emit_drain(c):
            """Move PV numerator+denominator out of PSUM, divide into ot."""
            b, qc = divmod(c, n_qc)
            qs = slice(qc * QC, (qc + 1) * QC)
            if qc == 0:
                ots[b] = ot_pool.tile([P, NQ], bf16, tag="ot", name="ot_sb")
            onum = num_pool.tile([P, 2, QC], f32, tag="onum", name="onum_t")
            # these two copies free the pv psum banks for chunk c+1
            for h in range(2):
                nc.vector.tensor_copy(onum[0:65, h, :], pvps[c][0:65, h, :])
            for h in range(2):
                drow = div_pool.tile([1, QC], f32, tag="drow", name="drow_t")
                nc.vector.tensor_copy(drow[:], onum[64:65, h, :])
                braw = div_pool.tile([64, QC], f32, tag="braw", name="braw_t")
                nc.gpsimd.partition_broadcast(braw[:], drow[:])
                rec = div_pool.tile([64, QC], f32, tag="rec", name="rec_t")
                nc.vector.reciprocal_approx_fast(rec[:], braw[:])
                nc.vector.tensor_mul(ots[b][64 * h:64 * (h + 1), qs],
                                     onum[0:64, h, :], rec[:])
            del pvps[c]
            del pts[c]

        def emit_a2a(b, h):
            """Reshard half h of batch b: dest core j gets its 128 tokens."""
            nc.gpsimd.dma_start(
                a2a_in[b][h].rearrange("(j p) t -> p j t", p=P),
                ots[b].rearrange("p (hh j t) -> p hh j t", hh=2, j=NCORES)[:, h])
            nc.gpsimd.collective_compute(
                "AllToAll", mybir.AluOpType.bypass,
                replica_groups=[list(range(NCORES))],
                ins=[a2a_in[b][h][:].opt()], outs=[a2a_out[b][h][:].opt()])

        def emit_at_loads(b, h):
            ats = []
            for cc in range(n_cc):
                at = at_pool.tile([P, P], bf16, tag="at", name="at_tile")
                nc.sync.dma_start(at[:],
                                  a2a_out[b][h][cc * P:(cc + 1) * P, :])
                ats.append(at)
            return ats

        def proj_unit(b, h, ats, oc):
            """One output-chunk of W_proj for half h of batch b (8 matmuls,
            self-contained tag-mm usage)."""
            def u():
                ocs = slice(oc * QC, (oc + 1) * QC)
                yps = ps_mm.tile([P, QC], f32, tag="mm", name="yps_t")
                for cc in range(n_cc):
                    nc.tensor.matmul(yps[:], ats[cc][:], wp_sb[:, cc, ocs],
                                     start=(cc == 0), stop=(cc == n_cc - 1))
                y_sb = y_pool.tile([P, QC], f32, tag="y", name="y_tile")
                nc.vector.tensor_add(y_sb[:], yps[:], bias_sb[:, ocs])
                nc.sync.dma_start(
                    out[(b * 2 + h) * TPH:(b * 2 + h + 1) * TPH, ocs],
                    y_sb[:])
            return u

        # ---------------- schedule ----------------
        # Ballast units (QK chains, proj output-chunks) are woven into the
        # step loop at fixed slots so the PE never idles while the exp stream
        # paces the chunk, and so collective-dependent proj matmuls enter the
        # in-order PE queue only well after their AllToAll completed.
        emit_xt(0, split=True)
        for u in qkv_units(0):  # batch 0's QKV as one block up front
            u()
        if NB > 1:
            emit_xt(1)
        nc.sync.dma_start(wp_sb[:], wp.rearrange("(cc p) m -> p cc m", p=P))
        nc.sync.dma_start(bias_row[:], bp[:, :])
        nc.gpsimd.partition_broadcast(bias_sb[:], bias_row[:])

        qk_pend = []        # pending QKV units for the next batch
        carry = {}          # chunk -> {k: unit} carried from earlier decisions
        ats_pend = {}
        OFF = 5             # pv steps trail scores by OFF steps
        for c in range(n_ck):
            b, qc = divmod(c, n_qc)
            if qc == 0:
                alloc_v(b)
            # build this chunk's unit slot map
            sl = dict(carry.pop(c, {}))
            if qc == 1:
                if b + 1 < NB:
                    qk_pend = qkv_units(b + 1)
                # batch 1's projection consumes the FIRST collective, which
                # completes later than steady-state ones — give it more cover
                pks = [8, 11] if b == 1 else [2, 5]
                uks = [2, 5, 14] if b == 1 else [8, 11, 14]
                if b >= 1:
                    ats = ats_pend.pop((b - 1, 0))
                    sl[pks[0]] = proj_unit(b - 1, 0, ats, 0)
                    sl[pks[1]] = proj_unit(b - 1, 0, ats, 1)
                for k in uks:                   # u0-u2 (need xt(b+1) cc 0-3)
                    if qk_pend:
                        sl[k] = qk_pend.pop(0)
            if qc == 2:
                for k in [2, 5, 8, 11, 14]:     # u3-u7
                    if qk_pend:
                        sl[k] = qk_pend.pop(0)
            if qc == 3:
                for k in [2, 5, 8, 11]:         # u8-u11
                    if qk_pend:
                        sl[k] = qk_pend.pop(0)
                if b >= 1:
                    ats = emit_at_loads(b - 1, 1)
                    sl[14] = proj_unit(b - 1, 1, ats, 0)
                    carry[c + 1] = {2: proj_unit(b - 1, 1, ats, 1)}
            # interleaved inner loop: pv of the previous chunk runs OFF steps
            # behind scores of this chunk; V matmuls of batch b are woven
            # into the batch's first chunk as ACT-independent ballast.
            for k in range(n_kt + OFF):
                if k >= OFF and c >= 1:
                    emit_pv_step(c - 1, k - OFF)
                if k < n_kt:
                    if qc == 0:
                        emit_v_tile(b, k)
                    if k in sl:
                        sl[k]()
                    emit_scores_step(c, k)
            if c >= 1:
                emit_drain(c - 1)
            # boundary work
            if qc == 0:
                if b >= 1:
                    emit_a2a(b - 1, 1)                  # needs drain(4b-1)
                    # their collective completed ~2 chunks ago
                    ats_pend[(b - 1, 0)] = emit_at_loads(b - 1, 0)
            if qc == 2:
                emit_a2a(b, 0)                          # needs drain(4b+1)
            if qc == 3 and b + 2 < NB:
                # x^T for batch b+2: batch b+1's QKV chains (its readers'
                # predecessors in the xt pool) have just finished with
                # xts[b+1], so the 4MB prefetch gets a full extra chunk of
                # DMA lead time before qkv_units(b+2) consume it
                emit_xt(b + 2)

        # epilogue: the last chunk's pv feeds the final AllToAll as fast as
        # possible — every deferrable projection runs AFTER the collective is
        # issued so it fills the ~20us collective latency instead of adding
        # to it (the collective starts only when the slowest core reaches it)
        last = n_ck - 1
        ats0 = emit_at_loads(NB - 1, 0)     # lands during pv(15)
        for k in range(n_kt):
            emit_pv_step(last, k)
        emit_drain(last)
        emit_a2a(NB - 1, 1)
        for u in carry.pop(n_ck, {}).values():
            u()
        proj_unit(NB - 1, 0, ats0, 0)()
        proj_unit(NB - 1, 0, ats0, 1)()
        ats1 = emit_at_loads(NB - 1, 1)
        proj_unit(NB - 1, 1, ats1, 0)()
        proj_unit(NB - 1, 1, ats1, 1)()

    nc.compile()
    return nc


def make_in_maps(x, W_qkv, W_proj, b_proj, NB=B, NQ=N, CH=C):
    """Shard the full inputs into one input map per core."""
    xT = np.ascontiguousarray(
        x.reshape(NB * NQ, CH).T).astype(BF16)
    wp = np.ascontiguousarray(W_proj).astype(BF16)
    bp = np.ascontiguousarray(b_proj[None, :]).astype(np.float32)
    ident = np.eye(P, dtype=BF16)
    in_maps = []
    for c in range(NCORES):
        cs = slice(P * c, P * (c + 1))
        in_maps.append({
            "xT": xT,
            "wq": np.ascontiguousarray(W_qkv[:, cs]).astype(BF16),
            "wk": np.ascontiguousarray(W_qkv[:, CH:][:, cs]).astype(BF16),
            "wv": np.ascontiguousarray(W_qkv[:, 2 * CH:][:, cs]).astype(BF16),
            "wp": wp,
            "bp": bp,
            "ident": ident,
        })
    return in_maps


def assemble_output(results, NB=B, NQ=N, CH=C):
    """Scatter the per-core half-batch token shards into the full output."""
    full = np.empty((NB, NQ, CH), dtype=np.float32)
    half = NQ // 2  # 1024
    for c in range(NCORES):
        y = np.asarray(results[c]["out"], dtype=np.float32)
        for b in range(NB):
            for h in range(2):
                dst = half * h + TPH * c
                src = (b * 2 + h) * TPH
                full[b, dst:dst + TPH, :] = y[src:src + TPH]
    return full


_compiled_nc = None


def kernel(x, W_qkv, W_proj, b_proj):
    global _compiled_nc
    x = np.asarray(x, dtype=np.float32)
    W_qkv = np.asarray(W_qkv, dtype=np.float32)
    W_proj = np.asarray(W_proj, dtype=np.float32)
    b_proj = np.asarray(b_proj, dtype=np.float32)

    if _compiled_nc is None:
        _compiled_nc = build_attention_nc()

    from concourse.bass_utils import run_bass_kernel_spmd

    in_maps = make_in_maps(x, W_qkv, W_proj, b_proj)
    res = run_bass_kernel_spmd(_compiled_nc, in_maps,
                               core_ids=list(range(NCORES)))
    return assemble_output(res.results)
